# revision 37
# baseline (speedup 1.0000x reference)
"""GAT (3-layer, PPI-style) Bass/Tile kernel for 8 Trainium2 NeuronCores.

Graph/data parallel by dst ownership (node shard of 6250/core). Per layer:
Phase A computes [feat|el|er] for owned nodes with one bf16 matmul against
W_aug = [W | W@al_bd | W@ar_bd]; a chunked AllGather publishes bf16
[feat|el] rows to every core; SWDGE dma_gather fetches per-edge rows by
src (int16 idxs, A/B half-tables); per-slot er comes from a small PE
matmul against a host-precomputed transposed one-hot (no er gather);
edge softmax + message aggregation run as one fused one-hot matmul per
128-slot tile with rhs [alpha*feat | alpha]; ELU + PE transpose produce
the next layer's x^T. Next layer's Phase A is emitted per-group right
after each group's edge phase so the AllGather overlaps edge work.

All graph-dependent index structures are computed on the host inside
kernel() and shipped as tensor inputs, so one SPMD program serves all
8 cores.
"""

import math

import numpy as np

P = 128
NCORES = 8


# ----------------------------------------------------------------------------
# Host-side preparation
# ----------------------------------------------------------------------------


def _wrap_idxs(idx, cols):
    """int16 idx array for dma_gather: wrapped in 16 partitions, replicated
    8x across 128 partitions. idx: [n] (n <= cols*16) -> [128, cols]."""
    n = idx.shape[0]
    arr = np.zeros(cols * 16, dtype=np.int16)
    arr[:n] = idx.astype(np.int16)
    w = arr.reshape(cols, 16).T  # [16, cols]
    return np.ascontiguousarray(np.tile(w, (8, 1)))  # [128, cols]


def _prepare(inputs):
    import ml_dtypes

    bf16 = ml_dtypes.bfloat16

    h = np.asarray(inputs["h"], dtype=np.float32)
    src = np.asarray(inputs["src"]).astype(np.int64)
    dst = np.asarray(inputs["dst"]).astype(np.int64)

    N, NFEAT = h.shape
    E = src.shape[0]
    assert N % NCORES == 0
    NOWN = N // NCORES
    G = math.ceil(NOWN / P)
    # X half: first NX nodes of each core (a balanced split; skewing X
    # larger pushes the X AllGather past its overlap window and regresses)
    NX = math.ceil(NOWN / (2 * P)) * P
    NY = NOWN - NX
    assert NCORES * NX <= 32767 and NCORES * NY <= 32767

    Ws, als, ars = [], [], []
    for i in (1, 2, 3):
        Ws.append(np.asarray(inputs[f"W{i}"], dtype=np.float32))
        als.append(np.asarray(inputs[f"al{i}"], dtype=np.float32))
        ars.append(np.asarray(inputs[f"ar{i}"], dtype=np.float32))
    H = als[0].shape[0]
    FEAT = [W.shape[1] for W in Ws]  # H*D per layer
    D = [f // H for f in FEAT]
    NCLASS = D[-1]

    Waug = []
    for W, al, ar, f, d in zip(Ws, als, ars, FEAT, D):
        al_bd = np.zeros((f, H), dtype=np.float32)
        ar_bd = np.zeros((f, H), dtype=np.float32)
        for hh in range(H):
            al_bd[hh * d : (hh + 1) * d, hh] = al[hh]
            ar_bd[hh * d : (hh + 1) * d, hh] = ar[hh]
        Waug.append(
            np.ascontiguousarray(
                np.concatenate([W, W @ al_bd, W @ ar_bd], axis=1)
            ).astype(bf16)
        )
    FO = [f + 2 * H for f in FEAT]
    # bf16 gather-table row widths (bytes multiple of 256 -> elems mult of 128)
    RW = [math.ceil((f + H) * 2 / 256) * 128 for f in FEAT]

    # ---- edge partitioning --------------------------------------------------
    owner = dst // NOWN
    per_core = []
    cntA = np.zeros((NCORES, G), dtype=np.int64)
    cntB = np.zeros((NCORES, G), dtype=np.int64)
    for c in range(NCORES):
        sel = np.nonzero(owner == c)[0]
        e_src = src[sel]
        dloc = dst[sel] - c * NOWN
        grp = dloc // P
        nloc = e_src % NOWN
        half = (nloc >= NX).astype(np.int64)
        order = np.lexsort((e_src, half, grp))
        e_src, dloc, grp, half = e_src[order], dloc[order], grp[order], half[order]
        for g in range(G):
            m = grp == g
            cntA[c, g] = int(np.count_nonzero(m & (half == 0)))
            cntB[c, g] = int(np.count_nonzero(m & (half == 1)))
        per_core.append((e_src, dloc, grp, half))

    SA = cntA.max(axis=0)  # static per-group gather counts (max over cores)
    SB = cntB.max(axis=0)
    kA = np.maximum(1, np.ceil(SA / P).astype(np.int64))
    kB = np.maximum(1, np.ceil(SB / P).astype(np.int64))
    SA = np.maximum(SA, 1)
    SB = np.maximum(SB, 1)
    K = kA + kB
    colsA = np.array([math.ceil(int(s) / 16) for s in SA], dtype=np.int64)
    colsB = np.array([math.ceil(int(s) / 16) for s in SB], dtype=np.int64)
    offcA = np.concatenate([[0], np.cumsum(colsA)])
    offcB = np.concatenate([[0], np.cumsum(colsB)])
    offT = np.concatenate([[0], np.cumsum(K)])  # tile offsets per group
    sumK = int(offT[-1])
    Kmax = int(K.max())

    in_maps = []
    for c in range(NCORES):
        e_src, dloc, grp, half = per_core[c]
        idxA = np.zeros((P, int(offcA[-1])), dtype=np.int16)
        idxB = np.zeros((P, int(offcB[-1])), dtype=np.int16)
        dstf = np.full((P, sumK), -1.0, dtype=np.float32)
        ohT = np.zeros((P, sumK * P), dtype=np.float32)
        pos = 0
        for g in range(G):
            nA = int(cntA[c, g])
            nB = int(cntB[c, g])
            eg = e_src[pos : pos + nA + nB]
            ks = eg // NOWN
            nl = eg % NOWN
            sA = (ks * NX + nl)[:nA]
            dA = dloc[pos : pos + nA] - g * P
            sB = (ks * NY + (nl - NX))[nA:]
            dB = dloc[pos + nA : pos + nA + nB] - g * P
            pos += nA + nB
            idxA[:, offcA[g] : offcA[g + 1]] = _wrap_idxs(sA, int(colsA[g]))
            idxB[:, offcB[g] : offcB[g + 1]] = _wrap_idxs(sB, int(colsB[g]))
            # slot space: kA[g] tiles of A edges, then kB[g] tiles of B
            dl = np.full(int(K[g]) * P, -1, dtype=np.int64)
            dl[:nA] = dA
            dl[int(kA[g]) * P : int(kA[g]) * P + nB] = dB
            dstf[:, offT[g] : offT[g + 1]] = (
                dl.reshape(int(K[g]), P).T.astype(np.float32)
            )
            o = (dl[None, :] == np.arange(P)[:, None]).astype(np.float32)
            ohT[:, offT[g] * P : offT[g + 1] * P] = o

        hT = np.ascontiguousarray(h[c * NOWN : (c + 1) * NOWN, :].T).astype(bf16)

        m = {
            "hT": hT,
            "iotaK": np.broadcast_to(
                np.tile(np.arange(P, dtype=np.float32), Kmax)[None, :],
                (P, Kmax * P),
            ).astype(bf16).copy(),
            "ident": np.eye(P, dtype=np.float32),
            "dstf": dstf.astype(bf16),
            "ohT": ohT.astype(bf16),
            "idxA": idxA,
            "idxB": idxB,
            "Wa1": Waug[0],
            "Wa2": Waug[1],
            "Wa3": Waug[2],
        }
        in_maps.append(m)

    cfg = dict(
        N=N,
        NFEAT=NFEAT,
        NOWN=NOWN,
        G=G,
        NX=NX,
        NY=NY,
        H=H,
        FEAT=tuple(FEAT),
        D=tuple(D),
        FO=tuple(FO),
        RW=tuple(RW),
        NCLASS=NCLASS,
        SA=tuple(int(x) for x in SA),
        SB=tuple(int(x) for x in SB),
        kA=tuple(int(x) for x in kA),
        kB=tuple(int(x) for x in kB),
        offcA=tuple(int(x) for x in offcA),
        offcB=tuple(int(x) for x in offcB),
        offT=tuple(int(x) for x in offT),
        Kmax=Kmax,
        kAmax=int(kA.max()),
        kBmax=int(kB.max()),
        sumK=sumK,
        colsA=tuple(int(x) for x in colsA),
        colsB=tuple(int(x) for x in colsB),
    )
    return cfg, in_maps


# ----------------------------------------------------------------------------
# Bass program
# ----------------------------------------------------------------------------


def _build(cfg):
    import concourse.bacc as bacc
    import concourse.mybir as mybir
    import concourse.tile as tile

    NOWN, G = cfg["NOWN"], cfg["G"]
    NX, NY = cfg["NX"], cfg["NY"]
    NFEAT, H = cfg["NFEAT"], cfg["H"]
    FEAT, FO, RW, D = cfg["FEAT"], cfg["FO"], cfg["RW"], cfg["D"]
    NCLASS = cfg["NCLASS"]
    SA, SB, kA, kB = cfg["SA"], cfg["SB"], cfg["kA"], cfg["kB"]
    offcA, offcB, offT = cfg["offcA"], cfg["offcB"], cfg["offT"]
    Kmax, kAmax, kBmax = cfg["Kmax"], cfg["kAmax"], cfg["kBmax"]
    sumK = cfg["sumK"]
    colsA, colsB = cfg["colsA"], cfg["colsB"]
    NEG = 0.2
    f32 = mybir.dt.float32
    bf16 = mybir.dt.bfloat16
    i16 = mybir.dt.int16
    AF = mybir.ActivationFunctionType
    OP = mybir.AluOpType

    F_IN = [NFEAT, FEAT[0], FEAT[1]]
    KT = [math.ceil(f / P) for f in F_IN]
    RWmax = max(RW)
    FOmax = max(FO)
    FTmax = max(FEAT)

    # AllGather node-range chunks (owned rows) and the phase-A group after
    # which each chunk's dispatch is emitted.
    nchunk = 4
    cb = [min(NOWN, math.ceil(NOWN / nchunk / P) * P * i) for i in range(nchunk + 1)]
    cb[-1] = NOWN
    disp_after = [min(G - 1, cb[i + 1] // P + 1) for i in range(nchunk)]
    disp_after[-1] = G - 1

    nc = bacc.Bacc(
        "TRN2", target_bir_lowering=False, debug=False, num_devices=NCORES
    )

    # ---- I/O ----------------------------------------------------------------
    hT_d = nc.dram_tensor("hT", [NFEAT, NOWN], bf16, kind="ExternalInput")
    iotaK_d = nc.dram_tensor("iotaK", [P, Kmax * P], bf16, kind="ExternalInput")
    ident_d = nc.dram_tensor("ident", [P, P], f32, kind="ExternalInput")
    dstf_d = nc.dram_tensor("dstf", [P, sumK], bf16, kind="ExternalInput")
    ohT_d = nc.dram_tensor("ohT", [P, sumK * P], bf16, kind="ExternalInput")
    idxA_d = nc.dram_tensor("idxA", [P, offcA[-1]], i16, kind="ExternalInput")
    idxB_d = nc.dram_tensor("idxB", [P, offcB[-1]], i16, kind="ExternalInput")
    W_d = [
        nc.dram_tensor(f"Wa{i + 1}", [F_IN[i], FO[i]], bf16, kind="ExternalInput")
        for i in range(3)
    ]
    out_d = nc.dram_tensor("out", [NOWN, NCLASS], f32, kind="ExternalOutput")

    ag_in = [
        nc.dram_tensor(f"ag_in{i}", [NOWN, RW[i]], bf16, kind="Internal")
        for i in range(3)
    ]
    tabX = [
        nc.dram_tensor(
            f"tabX{i}", [NCORES * NX, RW[i]], bf16, kind="Internal",
            addr_space="Shared",
        )
        for i in range(3)
    ]
    tabY = [
        nc.dram_tensor(
            f"tabY{i}", [NCORES * NY, RW[i]], bf16, kind="Internal",
            addr_space="Shared",
        )
        for i in range(3)
    ]

    rg = [list(range(NCORES))]

    with tile.TileContext(nc, num_cores=NCORES) as tc:
        with (
            tc.tile_pool(name="const", bufs=1) as cpool,
            tc.tile_pool(name="psum", bufs=1, space="PSUM") as pspool,
        ):
            iotaK_t = cpool.tile([P, Kmax * P], bf16, name="iotaK_t")
            ident_t = cpool.tile([P, P], f32, name="ident_t")
            dstf_t = cpool.tile([P, sumK], bf16, name="dstf_t")
            idxA_t = cpool.tile([P, offcA[-1]], i16, name="idxA_t")
            idxB_t = cpool.tile([P, offcB[-1]], i16, name="idxB_t")
            nc.sync.dma_start(iotaK_t[:], iotaK_d[:])
            nc.sync.dma_start(ident_t[:], ident_d[:])
            nc.sync.dma_start(dstf_t[:], dstf_d[:])
            nc.sync.dma_start(idxA_t[:], idxA_d[:])
            nc.sync.dma_start(idxB_t[:], idxB_d[:])

            W_t = []
            for l in range(3):
                slices = []
                for k in range(KT[l]):
                    r0 = k * P
                    r1 = min(r0 + P, F_IN[l])
                    w = cpool.tile([P, FO[l]], bf16, name=f"W{l}_{k}")
                    nc.sync.dma_start(w[: r1 - r0, :], W_d[l][r0:r1, :])
                    slices.append(w)
                W_t.append(slices)

            KTmax = max(KT)
            xT = [cpool.tile([P, NOWN], bf16, name=f"xT{k}") for k in range(KTmax)]
            for k in range(KT[0]):
                r0, r1 = k * P, min((k + 1) * P, NFEAT)
                nc.sync.dma_start(xT[k][: r1 - r0, :], hT_d[r0:r1, :])

            # double-buffered working tiles
            fbA12 = [cpool.tile([P, kAmax * RW[0]], bf16, name=f"fbA12_{i}") for i in range(3)]
            fbB12 = [cpool.tile([P, kBmax * RW[0]], bf16, name=f"fbB12_{i}") for i in range(3)]
            fbA3 = [cpool.tile([P, kAmax * RW[2]], bf16, name=f"fbA3_{i}") for i in range(3)]
            fbB3 = [cpool.tile([P, kBmax * RW[2]], bf16, name=f"fbB3_{i}") for i in range(3)]
            ohT_b = [cpool.tile([P, Kmax * P], bf16, name=f"ohT_{i}") for i in range(2)]
            oh_b = [cpool.tile([P, Kmax * P], bf16, name=f"oh_{i}") for i in range(2)]
            fs_b = [cpool.tile([P, Kmax * RWmax], bf16, name=f"fs_{i}") for i in range(2)]
            eef_b = [cpool.tile([P, Kmax * H], f32, name=f"eef_{i}") for i in range(2)]
            stage_b = [cpool.tile([P, RWmax], bf16, name=f"stage_{i}") for i in range(2)]
            er_b = [cpool.tile([P, G * H], bf16, name=f"er_{i}") for i in range(2)]
            sr_b = [cpool.tile([P, H], f32, name=f"sr_{i}") for i in range(2)]
            ss_b = [cpool.tile([P, H], f32, name=f"ss_{i}") for i in range(2)]
            xg_b = [cpool.tile([P, FTmax], f32, name=f"xg_{i}") for i in range(2)]
            mg_b = [cpool.tile([P, FTmax], f32, name=f"mg_{i}") for i in range(2)]
            o1_b = [cpool.tile([P, NCLASS], f32, name=f"o1_{i}") for i in range(2)]
            o2_b = [cpool.tile([P, NCLASS], f32, name=f"o2_{i}") for i in range(2)]

            psA = [pspool.tile([P, FOmax], f32, name=f"psA_{i}") for i in range(2)]
            ps = [pspool.tile([P, FTmax + H], f32, name=f"ps_{i}") for i in range(2)]
            pser = [pspool.tile([P, Kmax * H], f32, name=f"pser_{i}") for i in range(2)]
            pt = [pspool.tile([P, P], f32, name=f"pt_{i}") for i in range(2)]

            # prime buffers so never-written regions hold finite data (pad
            # slots of gathers; er partition rows beyond the last group's nn)
            for t in fbA12 + fbB12 + fbA3 + fbB3 + er_b:
                nc.vector.memset(t[:], 0.0)

            def phase_a(l, g):
                """feat/el/er for owned nodes of group g, layer l; stores
                [feat|el] bf16 rows to ag_in[l] and er to er_b[l % 2]."""
                nn = min(P, NOWN - g * P)
                pa = psA[g % 2]
                for k in range(KT[l]):
                    kk = min(P, F_IN[l] - k * P)
                    nc.tensor.matmul(
                        pa[:nn, 0 : FO[l]],
                        lhsT=xT[k][:kk, g * P : g * P + nn],
                        rhs=W_t[l][k][:kk, :],
                        start=(k == 0),
                        stop=(k == KT[l] - 1),
                    )
                st = stage_b[g % 2]
                nc.scalar.activation(
                    st[:nn, 0 : FEAT[l] + H], pa[:nn, 0 : FEAT[l] + H], AF.Copy
                )
                nc.scalar.activation(
                    er_b[l % 2][:nn, g * H : (g + 1) * H],
                    pa[:nn, FEAT[l] + H : FO[l]],
                    AF.Copy,
                )
                nc.sync.dma_start(
                    ag_in[l][g * P : g * P + nn, 0 : FEAT[l] + H],
                    st[:nn, 0 : FEAT[l] + H],
                )

            def dispatch_ag(l, which):
                if which == 0:
                    ins, outs = ag_in[l][0:NX, :], tabX[l][:]
                else:
                    ins, outs = ag_in[l][NX:NOWN, :], tabY[l][:]
                nc.gpsimd.collective_compute(
                    "AllGather",
                    mybir.AluOpType.bypass,
                    replica_groups=rg,
                    ins=[ins],
                    outs=[outs],
                )

            def edge(l, g):
                nn = min(P, NOWN - g * P)
                FT = FEAT[l]
                rw = RW[l]
                ka, kb_, Kg = kA[g], kB[g], kA[g] + kB[g]
                last = l == 2
                fA = (fbA3 if last else fbA12)[g % 3][:].rearrange(
                    "p (k r) -> p k r", r=rw
                )
                fB = (fbB3 if last else fbB12)[g % 3][:].rearrange(
                    "p (k r) -> p k r", r=rw
                )
                tabA = tabX[l][:]
                tabB = tabY[l][:]

                nc.gpsimd.dma_gather(
                    fA[:, 0:ka, :],
                    tabA,
                    idxA_t[:, offcA[g] : offcA[g] + colsA[g]],
                    SA[g],
                    SA[g],
                    rw,
                    elem_step=rw,
                    single_packet=(SA[g] <= 1008),
                )
                nc.gpsimd.dma_gather(
                    fB[:, 0:kb_, :],
                    tabB,
                    idxB_t[:, offcB[g] : offcB[g] + colsB[g]],
                    SB[g],
                    SB[g],
                    rw,
                    elem_step=rw,
                    single_packet=(SB[g] <= 1008),
                )

                oht = ohT_b[g % 2]
                nc.sync.dma_start(
                    oht[:, 0 : Kg * P], ohT_d[:, offT[g] * P : offT[g + 1] * P]
                )
                oh = oh_b[g % 2]
                nc.vector.tensor_tensor(
                    out=oh[:, 0 : Kg * P].rearrange("p (k x) -> p k x", x=P),
                    in0=dstf_t[:, offT[g] : offT[g + 1]].to_broadcast([P, Kg, P]),
                    in1=iotaK_t[:, 0 : Kg * P].rearrange("p (k x) -> p k x", x=P),
                    op=OP.is_equal,
                )

                # per-slot er via one-hot^T matmuls
                pe = pser[g % 2]
                for t in range(Kg):
                    nc.tensor.matmul(
                        pe[:, t * H : (t + 1) * H],
                        lhsT=oht[:, t * P : (t + 1) * P],
                        rhs=er_b[l % 2][:, g * H : (g + 1) * H],
                        start=True,
                        stop=True,
                    )

                # ee = exp(leaky_relu(el + er)), written strided into fs cols
                eef = eef_b[g % 2]
                fs = fs_b[g % 2][:].rearrange("p (k r) -> p k r", r=RWmax)
                nc.vector.tensor_add(
                    eef[:, 0 : ka * H].rearrange("p (k h) -> p k h", h=H),
                    fA[:, 0:ka, FT : FT + H],
                    pe[:, 0 : ka * H].rearrange("p (k h) -> p k h", h=H),
                )
                nc.vector.tensor_add(
                    eef[:, ka * H : Kg * H].rearrange("p (k h) -> p k h", h=H),
                    fB[:, 0:kb_, FT : FT + H],
                    pe[:, ka * H : Kg * H].rearrange("p (k h) -> p k h", h=H),
                )
                nc.vector.scalar_tensor_tensor(
                    out=eef[:, 0 : Kg * H],
                    in0=eef[:, 0 : Kg * H],
                    scalar=NEG,
                    in1=eef[:, 0 : Kg * H],
                    op0=OP.mult,
                    op1=OP.max,
                )
                nc.scalar.activation(
                    fs[:, 0:Kg, FT : FT + H],
                    eef[:, 0 : Kg * H].rearrange("p (k h) -> p k h", h=H),
                    AF.Exp,
                )

                # fs[:, :, 0:FT] = alpha_num * feat
                try:
                    nc.vector.tensor_mul(
                        fs[:, 0:ka, 0:FT].rearrange("p k (h d) -> p k h d", h=H),
                        fA[:, 0:ka, 0:FT].rearrange("p k (h d) -> p k h d", h=H),
                        fs[:, 0:ka, FT : FT + H].to_broadcast([P, ka, H, D[l]]),
                    )
                    nc.vector.tensor_mul(
                        fs[:, ka:Kg, 0:FT].rearrange("p k (h d) -> p k h d", h=H),
                        fB[:, 0:kb_, 0:FT].rearrange("p k (h d) -> p k h d", h=H),
                        fs[:, ka:Kg, FT : FT + H].to_broadcast([P, kb_, H, D[l]]),
                    )
                except Exception:
                    for t in range(Kg):
                        fsrc = fA[:, t, 0:FT] if t < ka else fB[:, t - ka, 0:FT]
                        nc.vector.tensor_mul(
                            fs[:, t, 0:FT].rearrange("p (h d) -> p h d", h=H),
                            fsrc.rearrange("p (h d) -> p h d", h=H),
                            fs[:, t, FT : FT + H].to_broadcast([P, H, D[l]]),
                        )

                po = ps[g % 2]
                for t in range(Kg):
                    nc.tensor.matmul(
                        po[:, 0 : FT + H],
                        lhsT=oh[:, t * P : (t + 1) * P],
                        rhs=fs[:, t, 0 : FT + H],
                        start=(t == 0),
                        stop=(t == Kg - 1),
                    )

                sr = sr_b[g % 2]
                ss = ss_b[g % 2]
                nc.scalar.activation(ss[:], po[:, FT : FT + H], AF.Copy)
                nc.vector.tensor_scalar_max(sr[:], ss[:], 1e-30)
                nc.vector.reciprocal(sr[:], sr[:])
                if last:
                    nc.vector.tensor_scalar_mul(sr[:], sr[:], 1.0 / H)
                xg = xg_b[g % 2]
                # xg[:, h-block] = po[:, h-block] * sr[:, h] on ACT
                # (per-partition scale AP), freeing DVE of the PSUM-read mul
                for hh in range(H):
                    nc.scalar.activation(
                        xg[:, hh * D[l] : (hh + 1) * D[l]],
                        po[:, hh * D[l] : (hh + 1) * D[l]],
                        AF.Copy,
                        scale=sr[:, hh : hh + 1],
                    )

                if not last:
                    mg = mg_b[g % 2]
                    nc.vector.tensor_scalar_min(mg[:, 0:FT], xg[:, 0:FT], 0.0)
                    nc.scalar.activation(mg[:, 0:FT], mg[:, 0:FT], AF.Exp)
                    nc.vector.scalar_tensor_tensor(
                        out=xg[:, 0:FT],
                        in0=mg[:, 0:FT],
                        scalar=-1.0,
                        in1=xg[:, 0:FT],
                        op0=OP.add,
                        op1=OP.max,
                    )
                    for kk in range(KT[l + 1]):
                        c0 = kk * P
                        c1 = min(c0 + P, FT)
                        w = c1 - c0
                        pt_ = pt[kk % 2]
                        nc.tensor.transpose(pt_[:w, :], xg[:, c0:c1], ident_t[:])
                        nc.scalar.activation(
                            xT[kk][:w, g * P : g * P + nn], pt_[:w, :nn], AF.Copy
                        )
                else:
                    o1 = o1_b[g % 2]
                    o2 = o2_b[g % 2]
                    nc.vector.tensor_add(
                        o1[:], xg[:, 0:NCLASS], xg[:, NCLASS : 2 * NCLASS]
                    )
                    nc.vector.tensor_add(
                        o2[:],
                        xg[:, 2 * NCLASS : 3 * NCLASS],
                        xg[:, 3 * NCLASS : 4 * NCLASS],
                    )
                    nc.vector.tensor_add(o1[:], o1[:], o2[:])
                    nc.sync.dma_start(out_d[g * P : g * P + nn, :], o1[:nn, :])

            # X rows [0:NX) are produced by phase-A groups [0, NX//P); the
            # X AllGather dispatches a few groups later and overlaps the
            # rest of the current phase; only the Y AllGather tail is
            # exposed at layer boundaries.
            gx = NX // P + 2

            # ---- prologue: layer-0 phase A + its AllGathers ----------------
            for g in range(G):
                phase_a(0, g)
                if g == gx:
                    dispatch_ag(0, 0)
            dispatch_ag(0, 1)

            # ---- main loop: edge(l) interleaved with phase_a(l+1) ----------
            for l in range(3):
                for g in range(G):
                    edge(l, g)
                    if l < 2:
                        phase_a(l + 1, g)
                        if g == gx:
                            dispatch_ag(l + 1, 0)
                if l < 2:
                    dispatch_ag(l + 1, 1)

    nc.compile()
    return nc


# ----------------------------------------------------------------------------
# Driver
# ----------------------------------------------------------------------------

_CACHE = {}


def _get_nc(cfg):
    key = str(sorted(cfg.items()))
    if key not in _CACHE:
        _CACHE[key] = _build(cfg)
    return _CACHE[key]


def _run(inputs, bench_iters=0, **_ignored):
    cfg, in_maps = _prepare(inputs)
    nc = _get_nc(cfg)
    outs, res = _pjrt_run(nc, in_maps, bench_iters=bench_iters)
    out = np.concatenate(outs, axis=0).astype(np.float32)
    return out, res


def _pjrt_run(nc, in_maps, bench_iters=0):
    """Execute the SPMD program on the 8 axon-tunneled cores via PJRT."""
    import time as _time

    import jax
    import numpy as _np
    from jax.sharding import Mesh, PartitionSpec
    from jax.experimental.shard_map import shard_map

    import concourse.mybir as mybir
    from concourse.bass2jax import (
        _bass_exec_p,
        install_neuronx_cc_hook,
        partition_id_tensor,
    )

    install_neuronx_cc_hook()
    n_cores = len(in_maps)

    partition_name = nc.partition_id_tensor.name if nc.partition_id_tensor else None
    in_names, out_names, out_avals, zero_outs = [], [], [], []
    for alloc in nc.m.functions[0].allocations:
        if not isinstance(alloc, mybir.MemoryLocationSet):
            continue
        name = alloc.memorylocations[0].name
        if alloc.kind == "ExternalInput":
            if name != partition_name:
                in_names.append(name)
        elif alloc.kind == "ExternalOutput":
            shape = tuple(alloc.tensor_shape)
            dtype = mybir.dt.np(alloc.dtype)
            out_names.append(name)
            out_avals.append(jax.core.ShapedArray(shape, dtype))
            zero_outs.append(_np.zeros(shape, dtype))
    n_params = len(in_names)
    n_outs = len(out_avals)
    in_names_all = list(in_names) + list(out_names)
    if partition_name is not None:
        in_names_all.append(partition_name)
    donate = tuple(range(n_params, n_params + n_outs))

    def _body(*args):
        operands = list(args)
        if partition_name is not None:
            operands.append(partition_id_tensor())
        outs = _bass_exec_p.bind(
            *operands,
            out_avals=tuple(out_avals),
            in_names=tuple(in_names_all),
            out_names=tuple(out_names),
            lowering_input_output_aliases=(),
            sim_require_finite=True,
            sim_require_nnan=True,
            nc=nc,
        )
        return tuple(outs)

    devices = jax.devices()[:n_cores]
    mesh = Mesh(_np.asarray(devices), ("core",))
    in_specs = (PartitionSpec("core"),) * (n_params + n_outs)
    out_specs = (PartitionSpec("core"),) * n_outs
    sharded = jax.jit(
        shard_map(
            _body, mesh=mesh, in_specs=in_specs, out_specs=out_specs,
            check_rep=False,
        ),
        donate_argnums=donate,
        keep_unused=True,
    )
    concat_in = [
        _np.concatenate([_np.asarray(in_maps[c][nm]) for c in range(n_cores)], axis=0)
        for nm in in_names
    ]

    def _zeros_dev():
        return [
            jax.device_put(
                _np.zeros((n_cores * z.shape[0], *z.shape[1:]), z.dtype),
                jax.sharding.NamedSharding(mesh, PartitionSpec("core")),
            )
            for z in zero_outs
        ]

    dev_in = [
        jax.device_put(a, jax.sharding.NamedSharding(mesh, PartitionSpec("core")))
        for a in concat_in
    ]

    out_arrs = sharded(*dev_in, *_zeros_dev())
    jax.block_until_ready(out_arrs)

    times = []
    for _ in range(bench_iters):
        zs = _zeros_dev()
        jax.block_until_ready(zs)
        t0 = _time.perf_counter()
        o = sharded(*dev_in, *zs)
        jax.block_until_ready(o)
        times.append(_time.perf_counter() - t0)

    outs = [
        {
            nm: _np.asarray(out_arrs[i]).reshape(n_cores, *out_avals[i].shape)[c]
            for i, nm in enumerate(out_names)
        }
        for c in range(n_cores)
    ]
    res = {"times_s": times, "min_time_ns": int(min(times) * 1e9) if times else None}
    return [o["out"] for o in outs], res


def kernel(**inputs):
    out, _ = _run(inputs)
    return out
